# revision 21
# baseline (speedup 1.0000x reference)
"""Multi-head self-attention on 8 Trainium2 NeuronCores.

Problem: B=4, S=2048, D=1024, H=16 heads (head_dim 64), fp32.
  out = softmax((x Wq + bq)(x Wk + bk)^T / 8) (x Wv + bv) Wo + bo

Sharding: 8 shards = 4 batches x 2 head-groups (8 heads each).
Core c handles batch c//2, heads (c%2)*8 .. (c%2)*8+8.  Wq/Wk/Wv are
column-sharded, Wo row-sharded; each core emits a partial [S, D] output
and the host sums the two partials per batch (the Wo all-reduce) + bo.

Per-core dataflow (all matmul operands bf16; PSUM accumulates fp32):
  x^T (host-pretransposed [D, S]) lives in SBUF, bf16.
  Q^T[dg,s], K^T[dg,s]: weight-stationary matmuls; K^T stays in SBUF.
  V[s,dg]: x-stationary matmuls, stored per head with a ones column ->
    the PV matmul also produces the softmax sums.
  Attention per (t=head-pair, b=si-block, hh): logits^T[sj,si] =
  (K^T chunk)^T Q^T, exp on ScalarE (scale=1/8, bf16 out; no max
  subtraction: logits ~ N(0,1)), P^T V via lhsT=[V|1].  Even heads
  write pv rows 0..64 (vals 0-63, sums 64); odd heads write rows
  63..127 (sums 63, vals 64-127) so vals land on their natural
  partitions with no partition-shift DMAs.
  Normalize: reciprocal of sums rows -> rr[1,1024]; build the
  broadcast matrix R[128,1024] with two accumulating PE matmuls
  (lhsT=e0/e1 [1,128] indicator rows); vals = pv * R + bv on DVE.
  vals^T is exactly the lhsT layout the output projection needs.
  Attention emission is software-pipelined (logits(sj+1) before
  PV(sj)) so the PE never waits on ScalarE exp.
"""
import numpy as np

B, S, D, H = 4, 2048, 1024, 16
HD = D // H          # 64
G = D // 2           # 512 columns per head-group
NCORES = 8
KT_ = 8              # D / 128 contraction tiles
TT = 4               # G / 128 dg tiles
ST = 16              # S / 128 s tiles
SB = 2               # si blocks
SBW = 1024           # si block width

_cache = {}


def _split_sync_waits(nc, mybir, max_waits=1):
    """walrus on this toolchain rejects >1 sem wait per instruction; move
    extra waits onto same-engine NoOps placed just before the instruction
    (engines are in-order, so this is semantics-preserving)."""
    for f in nc.m.functions:
        for bb in f.blocks:
            out, changed = [], False
            for inst in bb.instructions:
                si = inst.sync_info
                if si is not None and len(si.on_wait) > max_waits:
                    waits = list(si.on_wait)
                    head, tail = waits[:-max_waits], waits[-max_waits:]
                    for g in range(0, len(head), max_waits):
                        nop = mybir.InstNoOp(name=nc.get_next_instruction_name())
                        nop.engine = inst.engine
                        nop.sync_info = mybir.SyncInfo(
                            on_wait=head[g:g + max_waits], on_update=[])
                        nc.register_instruction(nop)
                        out.append(nop)
                    inst.sync_info = mybir.SyncInfo(
                        on_wait=tail, on_update=list(si.on_update))
                    changed = True
                out.append(inst)
            if changed:
                bb.instructions = out


def _build():
    import concourse.bass as bass
    import concourse.mybir as mybir
    import concourse.tile as tile

    F32 = mybir.dt.float32
    FR = mybir.dt.float32r
    BF = mybir.dt.bfloat16
    Exp = mybir.ActivationFunctionType.Exp

    nc = bass.Bass("TRN2", target_bir_lowering=False, debug=False,
                   num_devices=NCORES)
    xtd = nc.dram_tensor("xt", [D, S], BF, kind="ExternalInput")
    wqd = nc.dram_tensor("wq", [D, G], BF, kind="ExternalInput")
    wkd = nc.dram_tensor("wk", [D, G], BF, kind="ExternalInput")
    wvd = nc.dram_tensor("wv", [D, G], BF, kind="ExternalInput")
    wod = nc.dram_tensor("wo", [G, D], BF, kind="ExternalInput")
    bqd = nc.dram_tensor("bq", [G], F32, kind="ExternalInput")
    bkd = nc.dram_tensor("bk", [G], F32, kind="ExternalInput")
    bvd = nc.dram_tensor("bv", [G], F32, kind="ExternalInput")
    outd = nc.dram_tensor("out", [S, D], F32, kind="ExternalOutput")

    with tile.TileContext(nc) as tc, \
         nc.allow_low_precision(reason="bf16 matmul pipeline; rel-err budget 2e-2"), \
         tc.tile_pool(name="persist", bufs=1) as pp:
        if True:
            qts = pp.tile([128, TT, S], BF, tag="qts")
            kts = pp.tile([128, TT, S], BF, tag="kts")
            vsb = pp.tile([128, ST, 8, HD + 1], BF, tag="vsb")
            valsn = pp.tile([128, TT, S], BF, tag="valsn")
            wos = pp.tile([128, TT, D], BF, tag="wos")
            bqt = pp.tile([128, TT], F32, tag="bqt")
            bkt = pp.tile([128, TT], F32, tag="bkt")
            bvt = pp.tile([64, 8], F32, tag="bvt")

            nc.sync.dma_start(out=bqt, in_=bqd.rearrange("(t p) -> p t", p=128))
            nc.sync.dma_start(out=bkt, in_=bkd.rearrange("(t p) -> p t", p=128))
            nc.sync.dma_start(out=bvt, in_=bvd.rearrange("(h p) -> p h", p=64))
            nc.vector.memset(vsb[:, :, :, HD:HD + 1], 1.0)

            # ---- inputs: x^T on the SP DMA queue, weights on the ACT
            # queue (parallel rings); per-k tiles so matmuls pace with DMA.
            with tc.tile_pool(name="proj", bufs=1) as jp, \
                 tc.tile_pool(name="ppool", bufs=3) as ppl, \
                 tc.tile_pool(name="rrp", bufs=3) as rrp, \
                 tc.tile_pool(name="bcp", bufs=3) as bcp, \
                 tc.tile_pool(name="vtp", bufs=2) as vtp, \
                 tc.tile_pool(name="dram", bufs=1, space="DRAM") as dp, \
                 tc.tile_pool(name="ps_big", bufs=2, space="PSUM") as psb, \
                 tc.tile_pool(name="ps_pv", bufs=2, space="PSUM") as pspv:
                xts = [jp.tile([128, S], BF, tag=f"xts{k}", name=f"xts{k}")
                       for k in range(KT_)]
                wqs = [jp.tile([128, G], BF, tag=f"wqs{k}", name=f"wqs{k}")
                       for k in range(KT_)]
                wks = [jp.tile([128, G], BF, tag=f"wks{k}", name=f"wks{k}")
                       for k in range(KT_)]
                wvs = [jp.tile([128, G], BF, tag=f"wvs{k}", name=f"wvs{k}")
                       for k in range(KT_)]
                for k in range(KT_):
                    for h in range(2):
                        eng = nc.sync if (2 * k + h) % 2 == 0 else nc.gpsimd
                        eng.dma_start(
                            out=xts[k][:, h * 1024:(h + 1) * 1024],
                            in_=xtd[k * 128:(k + 1) * 128,
                                    h * 1024:(h + 1) * 1024])
                for k in range(KT_):
                    nc.scalar.dma_start(out=wvs[k], in_=wvd[k * 128:(k + 1) * 128, :])
                for k in range(KT_):
                    nc.scalar.dma_start(out=wqs[k], in_=wqd[k * 128:(k + 1) * 128, :])
                    nc.scalar.dma_start(out=wks[k], in_=wkd[k * 128:(k + 1) * 128, :])
                for t in range(TT):
                    nc.scalar.dma_start(out=wos[:, t, :], in_=wod[t * 128:(t + 1) * 128, :])

                # ---- Phase 1a: V projection (k-paced by the x^T DMAs) ----
                for s_ in range(ST):
                    ps = psb.tile([128, 512], F32, tag="big", name="pj")
                    for k in range(KT_):
                        nc.tensor.matmul(
                            ps, xts[k][:, s_ * 128:(s_ + 1) * 128],
                            wvs[k],
                            start=(k == 0), stop=(k == KT_ - 1))
                    nc.vector.tensor_copy(
                        out=vsb[:, s_, :, 0:HD],
                        in_=ps.rearrange("p (h d) -> p h d", h=8))

                # Q^T/K^T projection for one dg tile t (weight-stationary,
                # sc-sequential so only 2 accumulators are ever live).
                def emit_qk(t):
                    for ws, bt, dst in ((wqs, bqt, qts), (wks, bkt, kts)):
                        for sc in range(4):
                            ps = psb.tile([128, 512], F32, tag="big", name="pj")
                            for k in range(KT_):
                                nc.tensor.matmul(
                                    ps,
                                    ws[k][:, t * 128:(t + 1) * 128],
                                    xts[k][:, sc * 512:(sc + 1) * 512],
                                    start=(k == 0), stop=(k == KT_ - 1))
                            nc.vector.tensor_scalar_add(
                                dst[:, t, sc * 512:(sc + 1) * 512],
                                ps, bt[:, t:t + 1])

                # ---- Phase 2: attention (b outer so b0 norms finish early) --
                def emit_attention(t, b):
                    # software-pipelined emission: logits(i+1) before PV(i)
                    pvt = {}
                    pending = None  # (pv tile, pt tile, head, sj)

                    def emit_pv(pv, pt, h, sj):
                        lv = vsb[:, sj, h, 0:HD + 1]
                        for half in range(2):
                            nc.tensor.matmul(
                                pv[:, half * 512:(half + 1) * 512],
                                lv,
                                pt[:, half * 512:(half + 1) * 512],
                                start=(sj == 0), stop=(sj == ST - 1))

                    def emit_norm(hh):
                        # reciprocal of the sums row; broadcast across 64
                        # partitions on gpsimd (PE untouched), then
                        # scale + bias on DVE.
                        h = 2 * t + hh
                        rr = rrp.tile([1, SBW], F32, tag="rr")
                        nc.vector.reciprocal(out=rr, in_=pvt[hh][64:65, :])
                        # broadcast across partitions: bounce through DRAM
                        # (DMA can replicate a DRAM source; SBUF sources
                        # need nonzero partition step)
                        rrd = dp.tile([SBW], F32, tag="rrd", bufs=3)
                        nc.gpsimd.dma_start(
                            out=rrd.rearrange("(a b) -> a b", a=1), in_=rr)
                        bc = bcp.tile([64, SBW], F32, tag="bc")
                        nc.gpsimd.dma_start(
                            out=bc,
                            in_=rrd.rearrange("(a b) -> a b", a=1)
                                   .partition_broadcast(64))
                        bvcol = bvt[0:64, h:h + 1]
                        if hh == 0:
                            vn = valsn[0:64, t, b * SBW:(b + 1) * SBW]
                            nc.vector.tensor_mul(vn, pvt[0][0:64, :], bc)
                            nc.vector.tensor_scalar_add(vn, vn, bvcol)
                        else:
                            # DVE can't shift output partitions; compute at
                            # base 0 and DMA-shift into partitions 64:128.
                            vs = vtp.tile([64, SBW], BF, tag="vs")
                            nc.vector.tensor_mul(vs, pvt[1][0:64, :], bc)
                            nc.vector.tensor_scalar_add(vs, vs, bvcol)
                            nc.gpsimd.dma_start(
                                out=valsn[64:128, t, b * SBW:(b + 1) * SBW],
                                in_=vs)

                    for hh in range(2):
                        p0 = hh * 64
                        h = 2 * t + hh
                        qrow = qts[p0:p0 + 64, t, :]
                        pv = pspv.tile([65, SBW], F32, tag="pv")
                        pvt[hh] = pv
                        for sj in range(ST):
                            lg = psb.tile([128, SBW], F32, tag="big")
                            lkt = kts[p0:p0 + 64, t, sj * 128:(sj + 1) * 128]
                            for half in range(2):
                                nc.tensor.matmul(
                                    lg[:, half * 512:(half + 1) * 512],
                                    lkt,
                                    qrow[:, b * SBW + half * 512:
                                         b * SBW + (half + 1) * 512],
                                    start=True, stop=True)
                            pt = ppl.tile([128, SBW], BF, tag="pt")
                            nc.scalar.activation(pt, lg, Exp, scale=0.125)
                            if pending is not None:
                                emit_pv(*pending)
                            pending = (pv, pt, h, sj)
                        if hh == 0:
                            # drain head 0's last PV now so its normalize
                            # overlaps head 1's attention.
                            emit_pv(*pending)
                            pending = None
                            emit_norm(0)
                    emit_pv(*pending)
                    emit_norm(1)

                for t in range(TT):
                    emit_qk(t)
                    emit_attention(t, 0)
                for t in range(TT):
                    emit_attention(t, 1)

                # ---- Phase 3: output projection ----
                with tc.tile_pool(name="outp", bufs=3) as op_:
                    for s_ in range(ST):
                        s0 = s_ * 128
                        ops = psb.tile([128, SBW], F32, tag="big", name="ops")
                        for t in range(TT):
                            for half in range(2):
                                nc.tensor.matmul(
                                    ops[:, half * 512:(half + 1) * 512],
                                    valsn[:, t, s0:s0 + 128],
                                    wos[:, t, half * 512:(half + 1) * 512],
                                    start=(t == 0), stop=(t == TT - 1))
                        ob = op_.tile([128, D], F32, tag="ob")
                        nc.vector.tensor_copy(out=ob, in_=ops)
                        nc.sync.dma_start(out=outd[s0:s0 + 128, :], in_=ob)

    _split_sync_waits(nc, mybir)
    return nc


def _get_nc():
    if "nc" not in _cache:
        _cache["nc"] = _build()
    return _cache["nc"]


def _run(in_maps, **kw):
    from concourse.bass_utils import run_bass_kernel_spmd
    return run_bass_kernel_spmd(_get_nc(), in_maps, core_ids=list(range(NCORES)), **kw)


def _make_in_maps(x, Wq, bq, Wk, bk, Wv, bv, Wo, bo):
    import ml_dtypes
    BF = ml_dtypes.bfloat16
    x = np.asarray(x, np.float32)
    in_maps = []
    for c in range(NCORES):
        b, g = c // 2, c % 2
        gs = slice(g * G, (g + 1) * G)
        in_maps.append({
            "xt": np.ascontiguousarray(x[b].T).astype(BF),
            "wq": np.ascontiguousarray(np.asarray(Wq, np.float32)[:, gs]).astype(BF),
            "wk": np.ascontiguousarray(np.asarray(Wk, np.float32)[:, gs]).astype(BF),
            "wv": np.ascontiguousarray(np.asarray(Wv, np.float32)[:, gs]).astype(BF),
            "wo": np.ascontiguousarray(np.asarray(Wo, np.float32)[gs, :]).astype(BF),
            "bq": np.ascontiguousarray(np.asarray(bq, np.float32)[gs]),
            "bk": np.ascontiguousarray(np.asarray(bk, np.float32)[gs]),
            "bv": np.ascontiguousarray(np.asarray(bv, np.float32)[gs]),
        })
    return in_maps


def kernel(x, Wq, bq, Wk, bk, Wv, bv, Wo, bo, **_kw):
    res = _run(_make_in_maps(x, Wq, bq, Wk, bk, Wv, bv, Wo, bo))
    bo = np.asarray(bo, np.float32)
    out = np.empty((B, S, D), dtype=np.float32)
    for b in range(B):
        out[b] = res.results[2 * b]["out"] + res.results[2 * b + 1]["out"] + bo
    return out


# revision 25
# speedup vs baseline: 1.0039x; 1.0039x over previous
"""Multi-head self-attention on 8 Trainium2 NeuronCores.

Problem: B=4, S=2048, D=1024, H=16 heads (head_dim 64), fp32.
  out = softmax((x Wq + bq)(x Wk + bk)^T / 8) (x Wv + bv) Wo + bo

Sharding: 8 shards = 4 batches x 2 head-groups (8 heads each).
Core c handles batch c//2, heads (c%2)*8 .. (c%2)*8+8.  Wq/Wk/Wv are
column-sharded, Wo row-sharded; each core emits a partial [S, D] output
and the host sums the two partials per batch (the Wo all-reduce) + bo.

Per-core dataflow (all matmul operands bf16; PSUM accumulates fp32):
  x^T (host-pretransposed [D, S]) lives in SBUF, bf16; inputs stream
  on two parallel DMA queues (SP: x^T halves, ACT: weights) with
  per-k tiles so the first projections pace with the DMAs.
  Q^T[dg,s], K^T[dg,s]: weight-stationary matmuls; K^T stays in SBUF
  (no DRAM spill).  V[s,dg]: x-stationary matmuls, stored per head
  with a ones column -> the PV matmul also produces the softmax sums.
  Attention per (t=head-pair, b=si-block, hh): logits^T[sj,si] =
  (K^T chunk)^T Q^T, exp on ScalarE (scale=1/8, bf16 out; no max
  subtraction: logits ~ N(0,1)), P^T V via lhsT=[V|1] -> pv[65,1024]
  (vals rows 0-63, sums row 64).  Attention emission is
  software-pipelined (logits(sj+1) before PV(sj)) so the PE tracks
  the ScalarE exp stream with no PSUM ring stalls; the inner loop is
  exp-throughput-bound (1038ns/iter vs 852ns PE).
  Normalize (fully async, PE untouched): reciprocal of the sums row
  on DVE, partition-broadcast via a DRAM DMA bounce, then
  vals = pv * bc + bv on DVE.  Head 0 normalizes during head 1's
  attention; head 1's vals are computed at partition base 0 and
  DMA-shifted into partitions 64:127.  vals^T is exactly the lhsT
  layout the output projection needs.
  Loop order: b0 blocks (with QK(t) projections just-in-time),
  then b1 blocks back-to-back (exp-saturated), then the output
  projection through the same 2-slot PSUM ring.
  PSUM budget is the binding constraint: logits ring 2x[128,1024]
  (4 banks) + pv ring 2x[65,1024] (4 banks) = all 8 banks.
"""
import numpy as np

B, S, D, H = 4, 2048, 1024, 16
HD = D // H          # 64
G = D // 2           # 512 columns per head-group
NCORES = 8
KT_ = 8              # D / 128 contraction tiles
TT = 4               # G / 128 dg tiles
ST = 16              # S / 128 s tiles
SB = 2               # si blocks
SBW = 1024           # si block width

_cache = {}


def _split_sync_waits(nc, mybir, max_waits=1):
    """walrus on this toolchain rejects >1 sem wait per instruction; move
    extra waits onto same-engine NoOps placed just before the instruction
    (engines are in-order, so this is semantics-preserving)."""
    for f in nc.m.functions:
        for bb in f.blocks:
            out, changed = [], False
            for inst in bb.instructions:
                si = inst.sync_info
                if si is not None and len(si.on_wait) > max_waits:
                    waits = list(si.on_wait)
                    head, tail = waits[:-max_waits], waits[-max_waits:]
                    for g in range(0, len(head), max_waits):
                        nop = mybir.InstNoOp(name=nc.get_next_instruction_name())
                        nop.engine = inst.engine
                        nop.sync_info = mybir.SyncInfo(
                            on_wait=head[g:g + max_waits], on_update=[])
                        nc.register_instruction(nop)
                        out.append(nop)
                    inst.sync_info = mybir.SyncInfo(
                        on_wait=tail, on_update=list(si.on_update))
                    changed = True
                out.append(inst)
            if changed:
                bb.instructions = out


def _build():
    import concourse.bass as bass
    import concourse.mybir as mybir
    import concourse.tile as tile

    F32 = mybir.dt.float32
    BF = mybir.dt.bfloat16
    Exp = mybir.ActivationFunctionType.Exp

    nc = bass.Bass("TRN2", target_bir_lowering=False, debug=False,
                   num_devices=NCORES)
    xtd = nc.dram_tensor("xt", [D, S], BF, kind="ExternalInput")
    wqd = nc.dram_tensor("wq", [D, G], BF, kind="ExternalInput")
    wkd = nc.dram_tensor("wk", [D, G], BF, kind="ExternalInput")
    wvd = nc.dram_tensor("wv", [D, G], BF, kind="ExternalInput")
    wod = nc.dram_tensor("wo", [G, D], BF, kind="ExternalInput")
    bqd = nc.dram_tensor("bq", [G], F32, kind="ExternalInput")
    bkd = nc.dram_tensor("bk", [G], F32, kind="ExternalInput")
    bvd = nc.dram_tensor("bv", [G], F32, kind="ExternalInput")
    outd = nc.dram_tensor("out", [S, D], F32, kind="ExternalOutput")

    with tile.TileContext(nc) as tc, \
         nc.allow_low_precision(reason="bf16 matmul pipeline; rel-err budget 2e-2"), \
         tc.tile_pool(name="persist", bufs=1) as pp:
        if True:
            qts = pp.tile([128, TT, S], BF, tag="qts")
            kts = pp.tile([128, TT, S], BF, tag="kts")
            vsb = pp.tile([128, ST, 8, HD + 1], BF, tag="vsb")
            valsn = pp.tile([128, TT, S], BF, tag="valsn")
            wos = pp.tile([128, TT, D], BF, tag="wos")
            bqt = pp.tile([128, TT], F32, tag="bqt")
            bkt = pp.tile([128, TT], F32, tag="bkt")
            bvt = pp.tile([64, 8], F32, tag="bvt")

            nc.sync.dma_start(out=bqt, in_=bqd.rearrange("(t p) -> p t", p=128))
            nc.sync.dma_start(out=bkt, in_=bkd.rearrange("(t p) -> p t", p=128))
            nc.sync.dma_start(out=bvt, in_=bvd.rearrange("(h p) -> p h", p=64))
            nc.vector.memset(vsb[:, :, :, HD:HD + 1], 1.0)
            # warm the ScalarE Exp table during the DMA phase so the 1.3us
            # table load doesn't land inside the attention window.
            warm = pp.tile([1, 8], F32, tag="warm")
            nc.vector.memset(warm, 0.0)
            nc.scalar.activation(warm, warm, Exp)

            # ---- inputs: x^T on the SP DMA queue, weights on the ACT
            # queue (parallel rings); per-k tiles so matmuls pace with DMA.
            with tc.tile_pool(name="proj", bufs=1) as jp, \
                 tc.tile_pool(name="ppool", bufs=3) as ppl, \
                 tc.tile_pool(name="rrp", bufs=3) as rrp, \
                 tc.tile_pool(name="bcp", bufs=3) as bcp, \
                 tc.tile_pool(name="vtp", bufs=2) as vtp, \
                 tc.tile_pool(name="dram", bufs=1, space="DRAM") as dp, \
                 tc.tile_pool(name="ps_big", bufs=2, space="PSUM") as psb, \
                 tc.tile_pool(name="ps_pv", bufs=2, space="PSUM") as pspv:
                xts = [jp.tile([128, S], BF, tag=f"xts{k}", name=f"xts{k}")
                       for k in range(KT_)]
                wqs = [jp.tile([128, G], BF, tag=f"wqs{k}", name=f"wqs{k}")
                       for k in range(KT_)]
                wks = [jp.tile([128, G], BF, tag=f"wks{k}", name=f"wks{k}")
                       for k in range(KT_)]
                wvs = [jp.tile([128, G], BF, tag=f"wvs{k}", name=f"wvs{k}")
                       for k in range(KT_)]
                for k in range(KT_):
                    for h in range(2):
                        eng = nc.sync if (2 * k + h) % 2 == 0 else nc.gpsimd
                        eng.dma_start(
                            out=xts[k][:, h * 1024:(h + 1) * 1024],
                            in_=xtd[k * 128:(k + 1) * 128,
                                    h * 1024:(h + 1) * 1024])
                for k in range(KT_):
                    nc.scalar.dma_start(out=wvs[k], in_=wvd[k * 128:(k + 1) * 128, :])
                for k in range(KT_):
                    nc.scalar.dma_start(out=wqs[k], in_=wqd[k * 128:(k + 1) * 128, :])
                    nc.scalar.dma_start(out=wks[k], in_=wkd[k * 128:(k + 1) * 128, :])
                for t in range(TT):
                    nc.scalar.dma_start(out=wos[:, t, :], in_=wod[t * 128:(t + 1) * 128, :])

                # ---- Phase 1a: V projection (k-paced by the x^T DMAs) ----
                for s_ in range(ST):
                    ps = psb.tile([128, 512], F32, tag="big", name="pj")
                    for k in range(KT_):
                        nc.tensor.matmul(
                            ps, xts[k][:, s_ * 128:(s_ + 1) * 128],
                            wvs[k],
                            start=(k == 0), stop=(k == KT_ - 1))
                    nc.vector.tensor_copy(
                        out=vsb[:, s_, :, 0:HD],
                        in_=ps.rearrange("p (h d) -> p h d", h=8))

                # Q^T/K^T projection for one dg tile t (weight-stationary,
                # sc-sequential so only 2 accumulators are ever live).
                def emit_qk(t):
                    for ws, bt, dst in ((wqs, bqt, qts), (wks, bkt, kts)):
                        for sc in range(4):
                            ps = psb.tile([128, 512], F32, tag="big", name="pj")
                            for k in range(KT_):
                                nc.tensor.matmul(
                                    ps,
                                    ws[k][:, t * 128:(t + 1) * 128],
                                    xts[k][:, sc * 512:(sc + 1) * 512],
                                    start=(k == 0), stop=(k == KT_ - 1))
                            nc.vector.tensor_scalar_add(
                                dst[:, t, sc * 512:(sc + 1) * 512],
                                ps, bt[:, t:t + 1])

                # ---- Phase 2: attention (b outer so b0 norms finish early) --
                def emit_attention(t, b):
                    # software-pipelined emission: logits(i+1) before PV(i)
                    pvt = {}
                    pending = None  # (pv tile, pt tile, head, sj)

                    def emit_pv(pv, pt, h, sj):
                        lv = vsb[:, sj, h, 0:HD + 1]
                        for half in range(2):
                            nc.tensor.matmul(
                                pv[:, half * 512:(half + 1) * 512],
                                lv,
                                pt[:, half * 512:(half + 1) * 512],
                                start=(sj == 0), stop=(sj == ST - 1))

                    def emit_norm(hh):
                        # reciprocal of the sums row; broadcast across 64
                        # partitions on gpsimd (PE untouched), then
                        # scale + bias on DVE.
                        h = 2 * t + hh
                        rr = rrp.tile([1, SBW], F32, tag="rr")
                        nc.vector.reciprocal(out=rr, in_=pvt[hh][64:65, :])
                        # broadcast across partitions: bounce through DRAM
                        # (DMA can replicate a DRAM source; SBUF sources
                        # need nonzero partition step)
                        rrd = dp.tile([SBW], F32, tag="rrd", bufs=3)
                        nc.gpsimd.dma_start(
                            out=rrd.rearrange("(a b) -> a b", a=1), in_=rr)
                        bc = bcp.tile([64, SBW], F32, tag="bc")
                        nc.gpsimd.dma_start(
                            out=bc,
                            in_=rrd.rearrange("(a b) -> a b", a=1)
                                   .partition_broadcast(64))
                        bvcol = bvt[0:64, h:h + 1]
                        if hh == 0:
                            vn = valsn[0:64, t, b * SBW:(b + 1) * SBW]
                            nc.vector.tensor_mul(vn, pvt[0][0:64, :], bc)
                            nc.vector.tensor_scalar_add(vn, vn, bvcol)
                        else:
                            # DVE can't shift output partitions; compute at
                            # base 0 and DMA-shift into partitions 64:128.
                            vs = vtp.tile([64, SBW], BF, tag="vs")
                            nc.vector.tensor_mul(vs, pvt[1][0:64, :], bc)
                            nc.vector.tensor_scalar_add(vs, vs, bvcol)
                            nc.gpsimd.dma_start(
                                out=valsn[64:128, t, b * SBW:(b + 1) * SBW],
                                in_=vs)

                    for hh in range(2):
                        p0 = hh * 64
                        h = 2 * t + hh
                        qrow = qts[p0:p0 + 64, t, :]
                        pv = pspv.tile([65, SBW], F32, tag="pv")
                        pvt[hh] = pv
                        for sj in range(ST):
                            lg = psb.tile([128, SBW], F32, tag="big")
                            lkt = kts[p0:p0 + 64, t, sj * 128:(sj + 1) * 128]
                            for half in range(2):
                                nc.tensor.matmul(
                                    lg[:, half * 512:(half + 1) * 512],
                                    lkt,
                                    qrow[:, b * SBW + half * 512:
                                         b * SBW + (half + 1) * 512],
                                    start=True, stop=True)
                            pt = ppl.tile([128, SBW], BF, tag="pt")
                            nc.scalar.activation(pt, lg, Exp, scale=0.125)
                            if pending is not None:
                                emit_pv(*pending)
                            pending = (pv, pt, h, sj)
                        if hh == 0:
                            # drain head 0's last PV now so its normalize
                            # overlaps head 1's attention.
                            emit_pv(*pending)
                            pending = None
                            emit_norm(0)
                    emit_pv(*pending)
                    emit_norm(1)

                for t in range(TT):
                    emit_qk(t)
                    emit_attention(t, 0)
                for t in range(TT):
                    emit_attention(t, 1)

                # ---- Phase 3: output projection ----
                with tc.tile_pool(name="outp", bufs=3) as op_:
                    for s_ in range(ST):
                        s0 = s_ * 128
                        ops = psb.tile([128, SBW], F32, tag="big", name="ops")
                        ob = op_.tile([128, D], F32, tag="ob")
                        # half-major so each half evacuates while the other
                        # half's matmuls run (shorter ring hold + drain).
                        for half in range(2):
                            for t in range(TT):
                                nc.tensor.matmul(
                                    ops[:, half * 512:(half + 1) * 512],
                                    valsn[:, t, s0:s0 + 128],
                                    wos[:, t, half * 512:(half + 1) * 512],
                                    start=(t == 0), stop=(t == TT - 1))
                            nc.vector.tensor_copy(
                                out=ob[:, half * 512:(half + 1) * 512],
                                in_=ops[:, half * 512:(half + 1) * 512])
                            nc.sync.dma_start(
                                out=outd[s0:s0 + 128, half * 512:(half + 1) * 512],
                                in_=ob[:, half * 512:(half + 1) * 512])

    _split_sync_waits(nc, mybir)
    return nc


def _get_nc():
    if "nc" not in _cache:
        _cache["nc"] = _build()
    return _cache["nc"]


def _run(in_maps, **kw):
    from concourse.bass_utils import run_bass_kernel_spmd
    return run_bass_kernel_spmd(_get_nc(), in_maps, core_ids=list(range(NCORES)), **kw)


def _make_in_maps(x, Wq, bq, Wk, bk, Wv, bv, Wo, bo):
    import ml_dtypes
    BF = ml_dtypes.bfloat16
    x = np.asarray(x, np.float32)
    in_maps = []
    for c in range(NCORES):
        b, g = c // 2, c % 2
        gs = slice(g * G, (g + 1) * G)
        in_maps.append({
            "xt": np.ascontiguousarray(x[b].T).astype(BF),
            "wq": np.ascontiguousarray(np.asarray(Wq, np.float32)[:, gs]).astype(BF),
            "wk": np.ascontiguousarray(np.asarray(Wk, np.float32)[:, gs]).astype(BF),
            "wv": np.ascontiguousarray(np.asarray(Wv, np.float32)[:, gs]).astype(BF),
            "wo": np.ascontiguousarray(np.asarray(Wo, np.float32)[gs, :]).astype(BF),
            "bq": np.ascontiguousarray(np.asarray(bq, np.float32)[gs]),
            "bk": np.ascontiguousarray(np.asarray(bk, np.float32)[gs]),
            "bv": np.ascontiguousarray(np.asarray(bv, np.float32)[gs]),
        })
    return in_maps


def kernel(x, Wq, bq, Wk, bk, Wv, bv, Wo, bo, **_kw):
    res = _run(_make_in_maps(x, Wq, bq, Wk, bk, Wv, bv, Wo, bo))
    bo = np.asarray(bo, np.float32)
    out = np.empty((B, S, D), dtype=np.float32)
    for b in range(B):
        out[b] = res.results[2 * b]["out"] + res.results[2 * b + 1]["out"] + bo
    return out


# revision 34
# speedup vs baseline: 1.0196x; 1.0156x over previous
"""Multi-head self-attention on 8 Trainium2 NeuronCores.

Problem: B=4, S=2048, D=1024, H=16 heads (head_dim 64), fp32.
  out = softmax((x Wq + bq)(x Wk + bk)^T / 8) (x Wv + bv) Wo + bo

Sharding: 8 shards = 4 batches x 2 head-groups (8 heads each).
Core c handles batch c//2, heads (c%2)*8 .. (c%2)*8+8.  Wq/Wk/Wv are
column-sharded, Wo row-sharded; each core emits a partial [S, D] output
and the host sums the two partials per batch (the Wo all-reduce) + bo.

Per-core dataflow (all matmul operands bf16; PSUM accumulates fp32):
  x^T (host-pretransposed [D, S]) lives in SBUF, bf16; inputs stream
  on two parallel DMA queues (SP: x^T halves, ACT: weights) with
  per-k tiles so the first projections pace with the DMAs.
  Q^T[dg,s], K^T[dg,s]: weight-stationary matmuls; K^T stays in SBUF
  (no DRAM spill).  V[s,dg]: x-stationary matmuls, stored per head
  with a ones column -> the PV matmul also produces the softmax sums.
  Attention per (t=head-pair, b=si-block, hh): logits^T[sj,si] =
  (K^T chunk)^T Q^T, exp on ScalarE (scale=1/8, bf16 out; no max
  subtraction: logits ~ N(0,1)), P^T V via lhsT=[V|1] -> pv[65,1024]
  (vals rows 0-63, sums row 64).  Attention emission is
  software-pipelined (logits(sj+1) before PV(sj)) so the PE tracks
  the ScalarE exp stream with no PSUM ring stalls; the inner loop is
  exp-throughput-bound (1038ns/iter vs 852ns PE).
  Normalize (fully async, PE untouched): reciprocal of the sums row
  on DVE, partition-broadcast via a DRAM DMA bounce, then
  vals = pv * bc + bv on DVE.  Head 0 normalizes during head 1's
  attention; head 1's vals are computed at partition base 0 and
  DMA-shifted into partitions 64:127.  vals^T is exactly the lhsT
  layout the output projection needs.
  Loop order: b0 blocks (with QK(t) projections just-in-time),
  then b1 blocks back-to-back (exp-saturated), then the output
  projection through the same 2-slot PSUM ring.
  PSUM budget is the binding constraint: logits ring 2x[128,1024]
  (4 banks) + pv ring 2x[65,1024] (4 banks) = all 8 banks.
"""
import numpy as np

B, S, D, H = 4, 2048, 1024, 16
HD = D // H          # 64
G = D // 2           # 512 columns per head-group
NCORES = 8
KT_ = 8              # D / 128 contraction tiles
TT = 4               # G / 128 dg tiles
ST = 16              # S / 128 s tiles
SB = 2               # si blocks
SBW = 1024           # si block width

_cache = {}


def _split_sync_waits(nc, mybir, max_waits=1):
    """walrus on this toolchain rejects >1 sem wait per instruction; move
    extra waits onto same-engine NoOps placed just before the instruction
    (engines are in-order, so this is semantics-preserving)."""
    for f in nc.m.functions:
        for bb in f.blocks:
            out, changed = [], False
            for inst in bb.instructions:
                si = inst.sync_info
                if si is not None and len(si.on_wait) > max_waits:
                    waits = list(si.on_wait)
                    head, tail = waits[:-max_waits], waits[-max_waits:]
                    for g in range(0, len(head), max_waits):
                        nop = mybir.InstNoOp(name=nc.get_next_instruction_name())
                        nop.engine = inst.engine
                        nop.sync_info = mybir.SyncInfo(
                            on_wait=head[g:g + max_waits], on_update=[])
                        nc.register_instruction(nop)
                        out.append(nop)
                    inst.sync_info = mybir.SyncInfo(
                        on_wait=tail, on_update=list(si.on_update))
                    changed = True
                out.append(inst)
            if changed:
                bb.instructions = out


def _build():
    import concourse.bass as bass
    import concourse.mybir as mybir
    import concourse.tile as tile

    F32 = mybir.dt.float32
    BF = mybir.dt.bfloat16
    Exp = mybir.ActivationFunctionType.Exp

    nc = bass.Bass("TRN2", target_bir_lowering=False, debug=False,
                   num_devices=NCORES)
    xtd = nc.dram_tensor("xt", [D, S], BF, kind="ExternalInput")
    wqd = nc.dram_tensor("wq", [D, G], BF, kind="ExternalInput")
    wkd = nc.dram_tensor("wk", [D, G], BF, kind="ExternalInput")
    wvd = nc.dram_tensor("wv", [D, G], BF, kind="ExternalInput")
    wod = nc.dram_tensor("wo", [G, D], BF, kind="ExternalInput")
    bqd = nc.dram_tensor("bq", [G], F32, kind="ExternalInput")
    bkd = nc.dram_tensor("bk", [G], F32, kind="ExternalInput")
    bvd = nc.dram_tensor("bv", [G], F32, kind="ExternalInput")
    outd = nc.dram_tensor("out", [S, D], F32, kind="ExternalOutput")

    with tile.TileContext(nc) as tc, \
         nc.allow_low_precision(reason="bf16 matmul pipeline; rel-err budget 2e-2"), \
         tc.tile_pool(name="persist", bufs=1) as pp:
        if True:
            qts = pp.tile([128, TT, S], BF, tag="qts")
            kts = pp.tile([128, TT, S], BF, tag="kts")
            vsb = pp.tile([128, ST, 8, HD + 1], BF, tag="vsb")
            valsn = pp.tile([128, TT, S], BF, tag="valsn")
            wos = pp.tile([128, TT, D], BF, tag="wos")
            bqt = pp.tile([128, TT], F32, tag="bqt")
            bkt = pp.tile([128, TT], F32, tag="bkt")
            bvt = pp.tile([64, 8], F32, tag="bvt")

            nc.sync.dma_start(out=bqt, in_=bqd.rearrange("(t p) -> p t", p=128))
            nc.sync.dma_start(out=bkt, in_=bkd.rearrange("(t p) -> p t", p=128))
            nc.sync.dma_start(out=bvt, in_=bvd.rearrange("(h p) -> p h", p=64))
            nc.vector.memset(vsb[:, :, :, HD:HD + 1], 1.0)
            # warm the ScalarE Exp table during the DMA phase so the 1.3us
            # table load doesn't land inside the attention window.
            warm = pp.tile([1, 8], F32, tag="warm")
            nc.vector.memset(warm, 0.0)
            nc.scalar.activation(warm, warm, Exp)

            # ---- inputs: x^T on the SP DMA queue, weights on the ACT
            # queue (parallel rings); per-k tiles so matmuls pace with DMA.
            with tc.tile_pool(name="proj", bufs=1) as jp, \
                 tc.tile_pool(name="ppool", bufs=3) as ppl, \
                 tc.tile_pool(name="rrp", bufs=3) as rrp, \
                 tc.tile_pool(name="bcp", bufs=3) as bcp, \
                 tc.tile_pool(name="vtp", bufs=2) as vtp, \
                 tc.tile_pool(name="outp", bufs=3) as op_, \
                 tc.tile_pool(name="dram", bufs=1, space="DRAM") as dp, \
                 tc.tile_pool(name="ps_big", bufs=2, space="PSUM") as psb, \
                 tc.tile_pool(name="ps_pv", bufs=2, space="PSUM") as pspv:
                xts = [jp.tile([128, S], BF, tag=f"xts{k}", name=f"xts{k}")
                       for k in range(KT_)]
                wqs = [jp.tile([128, G], BF, tag=f"wqs{k}", name=f"wqs{k}")
                       for k in range(KT_)]
                wks = [jp.tile([128, G], BF, tag=f"wks{k}", name=f"wks{k}")
                       for k in range(KT_)]
                wvs = [jp.tile([128, G], BF, tag=f"wvs{k}", name=f"wvs{k}")
                       for k in range(KT_)]
                for k in range(KT_):
                    for h in range(2):
                        eng = nc.sync if (2 * k + h) % 2 == 0 else nc.gpsimd
                        eng.dma_start(
                            out=xts[k][:, h * 1024:(h + 1) * 1024],
                            in_=xtd[k * 128:(k + 1) * 128,
                                    h * 1024:(h + 1) * 1024])
                for k in range(KT_):
                    nc.scalar.dma_start(out=wvs[k], in_=wvd[k * 128:(k + 1) * 128, :])
                for k in range(KT_):
                    nc.scalar.dma_start(out=wqs[k], in_=wqd[k * 128:(k + 1) * 128, :])
                    nc.scalar.dma_start(out=wks[k], in_=wkd[k * 128:(k + 1) * 128, :])
                for t in range(TT):
                    nc.scalar.dma_start(out=wos[:, t, :], in_=wod[t * 128:(t + 1) * 128, :])

                # ---- Phase 1a: V projection (k-paced by the x^T DMAs) ----
                for s_ in range(ST):
                    ps = psb.tile([128, 512], F32, tag="big", name="pj")
                    for k in range(KT_):
                        nc.tensor.matmul(
                            ps, xts[k][:, s_ * 128:(s_ + 1) * 128],
                            wvs[k],
                            start=(k == 0), stop=(k == KT_ - 1))
                    nc.vector.tensor_copy(
                        out=vsb[:, s_, :, 0:HD],
                        in_=ps.rearrange("p (h d) -> p h d", h=8))

                # Q^T/K^T projection chains: 8 k-matmuls into one [128,512]
                # accumulator + bias-add evacuation.  Run either "solid"
                # (back-to-back through the big ring) or "fed" (spread one
                # matmul per attention iteration through the pv ring's idle
                # slot windows, filling the PE's exp-wait bubbles).
                def qk_chain(which, t, sc):
                    ws, bt, dst = ((wqs, bqt, qts) if which == "q"
                                   else (wks, bkt, kts))

                    def mk_mm(k):
                        def mm(ps):
                            nc.tensor.matmul(
                                ps,
                                ws[k][:, t * 128:(t + 1) * 128],
                                xts[k][:, sc * 512:(sc + 1) * 512],
                                start=(k == 0), stop=(k == KT_ - 1))
                        return mm

                    def evac(ps):
                        nc.vector.tensor_scalar_add(
                            dst[:, t, sc * 512:(sc + 1) * 512],
                            ps, bt[:, t:t + 1])
                    return [mk_mm(k) for k in range(KT_)], evac

                # output-projection half chains: 4 t-matmuls + copy + DMA.
                def op_chain(s_, half):
                    s0 = s_ * 128

                    def mk_mm(tt):
                        def mm(ps):
                            nc.tensor.matmul(
                                ps,
                                valsn[:, tt, s0:s0 + 128],
                                wos[:, tt, half * 512:(half + 1) * 512],
                                start=(tt == 0), stop=(tt == TT - 1))
                        return mm

                    def evac(ps):
                        ob = op_.tile([128, 512], F32, tag="ob", name="ob")
                        nc.vector.tensor_copy(out=ob, in_=ps)
                        nc.sync.dma_start(
                            out=outd[s0:s0 + 128, half * 512:(half + 1) * 512],
                            in_=ob)
                    return [mk_mm(tt) for tt in range(TT)], evac

                def chain_of(spec):
                    kind, a, b = spec
                    return (op_chain(a, b) if kind == "op"
                            else qk_chain(kind, a, b))

                def emit_solid(spec, pool=None):
                    mms, evac = chain_of(spec)
                    if pool is None:
                        pool = psb
                    tg = "big" if pool is psb else "pv"
                    ps = pool.tile([128, 512], F32, tag=tg, name="pj")
                    for mm in mms:
                        mm(ps)
                    evac(ps)

                # feeder: FIFO of chains drained one step per attention
                # iteration; each fed chain owns a [128,512] tile allocated
                # in the pv ring (allocation order per block must stay
                # pvt0, pvt1, chainA, chainB to keep slot parity).
                feed_fifo = []

                def feeder_push(spec, min_iter):
                    mms, evac = chain_of(spec)
                    ps = pspv.tile([128, 512], F32, tag="pv", name="ch")
                    steps = [(lambda mm=mm, ps=ps: mm(ps)) for mm in mms]
                    steps.append(lambda evac=evac, ps=ps: evac(ps))
                    feed_fifo.append({"steps": steps, "i": 0,
                                      "min_iter": min_iter})

                def feeder_tick(it):
                    if not feed_fifo:
                        return
                    head = feed_fifo[0]
                    if it < head["min_iter"]:
                        return
                    head["steps"][head["i"]]()
                    head["i"] += 1
                    if head["i"] == len(head["steps"]):
                        feed_fifo.pop(0)

                # ---- Phase 2: attention ----
                def emit_attention(t, b, push_a=None):
                    # software-pipelined emission: logits(i+1) before PV(i)
                    pvt = {}
                    pending = None  # (pv tile, pt tile, head, sj)

                    def emit_pv(pv, pt, h, sj):
                        lv = vsb[:, sj, h, 0:HD + 1]
                        for half in range(2):
                            nc.tensor.matmul(
                                pv[:, half * 512:(half + 1) * 512],
                                lv,
                                pt[:, half * 512:(half + 1) * 512],
                                start=(sj == 0), stop=(sj == ST - 1))

                    def emit_norm(hh):
                        # reciprocal of the sums row; broadcast across 64
                        # partitions on gpsimd (PE untouched), then
                        # scale + bias on DVE.
                        h = 2 * t + hh
                        rr = rrp.tile([1, SBW], F32, tag="rr")
                        nc.vector.reciprocal(out=rr, in_=pvt[hh][64:65, :])
                        # broadcast across partitions: bounce through DRAM
                        # (DMA can replicate a DRAM source; SBUF sources
                        # need nonzero partition step)
                        rrd = dp.tile([SBW], F32, tag="rrd", bufs=3)
                        nc.gpsimd.dma_start(
                            out=rrd.rearrange("(a b) -> a b", a=1), in_=rr)
                        bc = bcp.tile([64, SBW], F32, tag="bc")
                        nc.gpsimd.dma_start(
                            out=bc,
                            in_=rrd.rearrange("(a b) -> a b", a=1)
                                   .partition_broadcast(64))
                        bvcol = bvt[0:64, h:h + 1]
                        if hh == 0:
                            vn = valsn[0:64, t, b * SBW:(b + 1) * SBW]
                            nc.vector.tensor_mul(vn, pvt[0][0:64, :], bc)
                            nc.vector.tensor_scalar_add(vn, vn, bvcol)
                        else:
                            # DVE can't shift output partitions; compute at
                            # base 0 and DMA-shift into partitions 64:128.
                            vs = vtp.tile([64, SBW], BF, tag="vs")
                            nc.vector.tensor_mul(vs, pvt[1][0:64, :], bc)
                            nc.vector.tensor_scalar_add(vs, vs, bvcol)
                            nc.gpsimd.dma_start(
                                out=valsn[64:128, t, b * SBW:(b + 1) * SBW],
                                in_=vs)

                    for hh in range(2):
                        p0 = hh * 64
                        h = 2 * t + hh
                        qrow = qts[p0:p0 + 64, t, :]
                        pv = pspv.tile([65, SBW], F32, tag="pv")
                        pvt[hh] = pv
                        if hh == 1 and push_a is not None:
                            push_a()
                        for sj in range(ST):
                            lg = psb.tile([128, SBW], F32, tag="big")
                            lkt = kts[p0:p0 + 64, t, sj * 128:(sj + 1) * 128]
                            for half in range(2):
                                nc.tensor.matmul(
                                    lg[:, half * 512:(half + 1) * 512],
                                    lkt,
                                    qrow[:, b * SBW + half * 512:
                                         b * SBW + (half + 1) * 512],
                                    start=True, stop=True)
                            pt = ppl.tile([128, SBW], BF, tag="pt")
                            nc.scalar.activation(pt, lg, Exp, scale=0.125)
                            if pending is not None:
                                emit_pv(*pending)
                            pending = (pv, pt, h, sj)
                            feeder_tick(sj)
                        if hh == 0:
                            # drain head 0's last PV now so its normalize
                            # overlaps head 1's attention.
                            emit_pv(*pending)
                            pending = None
                            emit_norm(0)
                    emit_pv(*pending)
                    emit_norm(1)

                # static feed schedule.  Boundary i's pair: chainA drains
                # during block i's hh1 (usable from block i+1), chainB during
                # block i+1's hh0 (usable late block i+1 / block i+2).
                # K sc3 is only read from iteration 12, so it tolerates the
                # chainB landing; q sc0/1 feed b0 blocks, sc2/3 b1 blocks.
                boundary_pairs = {
                    1: [("q", 1, 0, 5), ("k", 1, 0, 3)],
                    2: [("q", 1, 1, 5), ("k", 1, 3, 3)],
                    3: [("q", 1, 2, 5), ("k", 2, 0, 3)],
                    4: [("q", 2, 0, 5), ("k", 2, 3, 3)],
                    5: [("q", 2, 2, 5), ("k", 3, 0, 3)],
                    6: [("q", 3, 0, 5), ("k", 3, 3, 3)],
                    7: [("q", 3, 2, 5), ("op", 0, 0, 3)],
                    8: [("op", 0, 1, 5)],  # A only: no block follows
                }
                solids_before = {
                    1: [("q", 0, sc) for sc in range(4)] +
                       [("k", 0, sc) for sc in range(4)],
                    3: [("k", 1, 1), ("k", 1, 2)],
                    4: [("q", 1, 3)],
                    5: [("q", 2, 1), ("k", 2, 1), ("k", 2, 2)],
                    6: [("q", 2, 3)],
                    7: [("q", 3, 1), ("k", 3, 1), ("k", 3, 2)],
                    8: [("q", 3, 3)],
                }

                blocks = [(t, b) for t in range(TT) for b in range(SB)]
                for bi, (t, b) in enumerate(blocks, start=1):
                    for spec in solids_before.get(bi, []):
                        emit_solid(spec)
                    pair = boundary_pairs.get(bi)
                    emit_attention(
                        t, b,
                        push_a=(lambda p=pair: feeder_push(p[0][:3], p[0][3]))
                        if pair else None)
                    if pair and len(pair) > 1:
                        feeder_push(pair[1][:3], pair[1][3])
                assert not feed_fifo, f"undrained chains: {len(feed_fifo)}"

                # ---- Phase 3: remaining output projection (solid) ----
                # alternate both PSUM rings (pv ring is free now) so the
                # 4-deep pipeline hides the copy+DMA evacuations.
                fed_ops = {(0, 0), (0, 1)}
                for s_ in range(ST):
                    for half in range(2):
                        if (s_, half) not in fed_ops:
                            emit_solid(("op", s_, half))

    _split_sync_waits(nc, mybir)
    return nc


def _get_nc():
    if "nc" not in _cache:
        _cache["nc"] = _build()
    return _cache["nc"]


def _run(in_maps, **kw):
    from concourse.bass_utils import run_bass_kernel_spmd
    return run_bass_kernel_spmd(_get_nc(), in_maps, core_ids=list(range(NCORES)), **kw)


def _make_in_maps(x, Wq, bq, Wk, bk, Wv, bv, Wo, bo):
    import ml_dtypes
    BF = ml_dtypes.bfloat16
    x = np.asarray(x, np.float32)
    in_maps = []
    for c in range(NCORES):
        b, g = c // 2, c % 2
        gs = slice(g * G, (g + 1) * G)
        in_maps.append({
            "xt": np.ascontiguousarray(x[b].T).astype(BF),
            "wq": np.ascontiguousarray(np.asarray(Wq, np.float32)[:, gs]).astype(BF),
            "wk": np.ascontiguousarray(np.asarray(Wk, np.float32)[:, gs]).astype(BF),
            "wv": np.ascontiguousarray(np.asarray(Wv, np.float32)[:, gs]).astype(BF),
            "wo": np.ascontiguousarray(np.asarray(Wo, np.float32)[gs, :]).astype(BF),
            "bq": np.ascontiguousarray(np.asarray(bq, np.float32)[gs]),
            "bk": np.ascontiguousarray(np.asarray(bk, np.float32)[gs]),
            "bv": np.ascontiguousarray(np.asarray(bv, np.float32)[gs]),
        })
    return in_maps


def kernel(x, Wq, bq, Wk, bk, Wv, bv, Wo, bo, **_kw):
    res = _run(_make_in_maps(x, Wq, bq, Wk, bk, Wv, bv, Wo, bo))
    bo = np.asarray(bo, np.float32)
    out = np.empty((B, S, D), dtype=np.float32)
    for b in range(B):
        out[b] = res.results[2 * b]["out"] + res.results[2 * b + 1]["out"] + bo
    return out
